# revision 4
# baseline (speedup 1.0000x reference)
"""Block-diagonal MLP kernel for TRN2, 8 NeuronCores.

Computes out = x @ tanh(blocks * mask) where blocks is 4096x4096 with 16
diagonal 256x256 blocks (mask is the fixed block-diagonal pattern). Only the
diagonal blocks matter (tanh(0)=0):

    out[:, 256k:256(k+1)] = x[:, 256k:256(k+1)] @ tanh(B_k)

Sharding: block-parallel. Core c owns blocks 2c and 2c+1 (512 contiguous
k/n-columns) and streams all 8192 rows of x:

    outT_shard[n, m] = sum_k b[k, n] * xT_shard[k, m]      (n, k local to core)

The kernel is DMA-bound (16 DMA engines x ~26 GB/s/core), so both streams
ship reduced: x goes to the device as bf16 pre-scaled by 1/8 on the host
(exact: exponent shift), and the output comes back as float8 e3m4 holding
out/8 (range +-7.7, fits e3m4's +-15.5; host decodes *8). That halves the
store stream vs bf16. Weights ship pre-packed on the host into the exact
SBUF lhsT layout as bf16 (contiguous 0.26MB DMA) and tanh runs on-device in
256-col chunks so the first LDWEIGHTS can issue as early as possible.

Matmul schedule: weight-stationary phases (blk, ncol), each streaming all
8192 m in 2048-col PSUM tiles (4 banks, 2-tile rotation = all 8 banks).
Each PSUM tile takes 8 matmuls (kc 0/1 accumulation x 4 chunks of 512) with
only 2 weight switches, then is evacuated split: DVE takes cols 0:1024 and
ACT takes 1024:2048 concurrently (one engine's evac stream cannot keep pace
with the PE). Stores of [128, 4096] fp8 tiles alternate between the ACT and
DVE HWDGE rings; loads own the Sync ring.

Measured rel_l2 vs f32 reference: ~1.35e-2 (dominated by the e3m4 output
rounding; gate is 2e-2).
"""

import ml_dtypes
import numpy as np

import concourse.mybir as mybir
import concourse.tile as tile
from concourse import bacc
from concourse.bass_utils import run_bass_kernel_spmd

N_CORES = 8
N_ROWS = 8192            # rows of x / out
D = 4096                 # layer size
BLOCK = 256              # block size
BLOCKS_PER_CORE = 2      # 16 blocks / 8 cores
K_PER_CORE = BLOCKS_PER_CORE * BLOCK   # 512 k (and n) columns per core
MM_FREE = 512            # matmul moving free dim (one fp32 PSUM bank)
MSEG = 2048              # PSUM tile free dim (4 banks)
N_MSEG = N_ROWS // MSEG  # 4
HALF = 4096              # x-tile free dim / store granularity
OUT_SCALE = 8.0          # host folds x/8 in, decodes out*8

_nc_cache = None


def _build_nc():
    f32 = mybir.dt.float32
    bf16 = mybir.dt.bfloat16
    e3 = mybir.dt.float8e3

    # Bacc (not Bass): its compile() runs move_matmul_waits_to_ldweights and
    # generate_event_semaphores (splits multi-sem waits down to the 1
    # sync-wait-per-instruction the hardware supports).
    nc = bacc.Bacc("TRN2")
    xT = nc.dram_tensor("xT", [K_PER_CORE, N_ROWS], bf16, kind="ExternalInput")
    wp = nc.dram_tensor("wp", [128, 1024], bf16, kind="ExternalInput")
    outT = nc.dram_tensor("outT", [K_PER_CORE, N_ROWS], e3, kind="ExternalOutput")

    with tile.TileContext(nc) as tc:
        with (
            tc.tile_pool(name="bpool", bufs=1) as bpool,
            tc.tile_pool(name="xpool", bufs=8) as xpool,
            tc.tile_pool(name="opool", bufs=4) as opool,
            tc.tile_pool(name="pspool", bufs=2, space="PSUM") as pspool,
        ):
            # --- weights: single contiguous DMA of the host-packed lhsT
            # layout; col chunk (blk*2+kc)*256 + n holds
            # blocks[k0+blk*256+kc*128+p, k0+blk*256+n]
            w_raw = bpool.tile([128, 1024], bf16, name="w_raw")
            b_mm = bpool.tile([128, 1024], bf16, name="b_mm")
            nc.sync.dma_start(out=w_raw[:], in_=wp[:])

            # --- x loads on the Sync ring, in consumption order
            xts = {}
            for q, h in [(0, 0), (1, 0), (0, 1), (1, 1),
                         (2, 0), (3, 0), (2, 1), (3, 1)]:
                t = xpool.tile([128, HALF], bf16, name=f"x{q}_{h}", tag="xt")
                nc.sync.dma_start(
                    out=t[:],
                    in_=xT[q * 128:(q + 1) * 128, h * HALF:(h + 1) * HALF],
                )
                xts[(q, h)] = t

            # tanh in 256-col chunks: the first chunk covers (blk0, kc0) and
            # unblocks the first LDWEIGHTS right after the ACT table load
            for c in range(4):
                nc.scalar.activation(
                    b_mm[:, c * 256:(c + 1) * 256],
                    w_raw[:, c * 256:(c + 1) * 256],
                    mybir.ActivationFunctionType.Tanh,
                )

            # --- weight-stationary matmul phases ---

            for blk in range(BLOCKS_PER_CORE):
                for ncol in range(2):  # n chunk of 128 within the block
                    r0 = blk * BLOCK + ncol * 128
                    osb = None
                    for ms in range(N_MSEG):
                        h = ms // 2   # which 4096-col half of m
                        hh = ms % 2   # position within that half
                        if hh == 0:
                            osb = opool.tile([128, HALF], e3, name="osb",
                                             tag="osb")
                        ps = pspool.tile([128, MSEG], f32, name="ps")
                        for kc in range(2):
                            q = blk * 2 + kc
                            lcol = q * 256 + ncol * 128
                            lhsT = b_mm[:, lcol:lcol + 128]
                            for mi in range(MSEG // MM_FREE):
                                mlo = ms * MSEG + mi * MM_FREE
                                xt = xts[(q, mlo // HALF)]
                                xl = mlo % HALF
                                nc.tensor.matmul(
                                    ps[:, mi * MM_FREE:(mi + 1) * MM_FREE],
                                    lhsT=lhsT,
                                    rhs=xt[:, xl:xl + MM_FREE],
                                    start=(kc == 0),
                                    stop=(kc == 1),
                                )
                        # evac split: DVE drains cols 0:1024, ACT 1024:2048,
                        # concurrently (single-engine drain can't keep PE pace)
                        d0 = hh * MSEG
                        nc.vector.tensor_copy(osb[:, d0:d0 + 1024],
                                              ps[:, 0:1024])
                        nc.scalar.copy(osb[:, d0 + 1024:d0 + 2048],
                                       ps[:, 1024:2048])
                        if hh == 1:
                            # stores on the ACT HWDGE ring: separate queue
                            # from the Sync-ring loads (only SP and ACT have
                            # HWDGE rings on TRN2)
                            dst = outT[r0:r0 + 128, h * HALF:(h + 1) * HALF]
                            nc.scalar.dma_start(out=dst, in_=osb[:])
    nc.compile()
    return nc


def _get_nc():
    global _nc_cache
    if _nc_cache is None:
        _nc_cache = _build_nc()
    return _nc_cache


def _make_in_maps(x, blocks):
    bf = ml_dtypes.bfloat16
    # x/8 is exact in bf16 (exponent shift); the device computes out/8 so the
    # e3m4 output wire never saturates (|out/8| < 7.7 vs e3m4 max 15.5)
    xT = (x.T / OUT_SCALE).astype(bf)  # [4096, 8192]
    in_maps = []
    for c in range(N_CORES):
        k0 = c * K_PER_CORE
        cols = []
        for blk in range(BLOCKS_PER_CORE):
            for kc in range(2):
                rlo = k0 + blk * BLOCK + kc * 128
                cols.append(blocks[rlo:rlo + 128,
                                   k0 + blk * BLOCK:k0 + (blk + 1) * BLOCK])
        wpk = np.ascontiguousarray(np.concatenate(cols, axis=1)).astype(bf)
        in_maps.append({
            "xT": np.ascontiguousarray(xT[k0:k0 + K_PER_CORE]),
            "wp": wpk,
        })
    return in_maps


def _run(x, blocks, **spmd_kwargs):
    res = run_bass_kernel_spmd(
        _get_nc(), _make_in_maps(x, blocks), core_ids=list(range(N_CORES)),
        **spmd_kwargs,
    )
    out = np.empty((N_ROWS, D), np.float32)
    for c in range(N_CORES):
        shard = res.results[c]["outT"].astype(np.float32) * OUT_SCALE
        out[:, c * K_PER_CORE:(c + 1) * K_PER_CORE] = shard.T
    return out, res


def kernel(x, blocks, mask=None):
    out, _ = _run(np.asarray(x), np.asarray(blocks))
    return out


# revision 5
# speedup vs baseline: 1.2052x; 1.2052x over previous
"""Block-diagonal MLP kernel for TRN2, 8 NeuronCores.

Computes out = x @ tanh(blocks * mask) where blocks is 4096x4096 with 16
diagonal 256x256 blocks (mask is the fixed block-diagonal pattern). Only the
diagonal blocks matter (tanh(0)=0):

    out[:, 256k:256(k+1)] = x[:, 256k:256(k+1)] @ tanh(B_k)

Sharding: block-parallel. Core c owns blocks 2c and 2c+1 (512 contiguous
k/n-columns) and streams all 8192 rows of x:

    outT_shard[n, m] = sum_k b[k, n] * xT_shard[k, m]      (n, k local to core)

Wire formats (DMA is the co-bottleneck with the PE): x ships bf16 pre-scaled
by 1/8 on the host (exact: exponent shift); the output ships as float8 e3m4
holding out/8 (|out/8| < 7.7 vs e3m4 max 15.5; host decodes *8), halving the
store stream vs bf16. Weights ship host-packed in the exact SBUF lhsT layout
as bf16 (one contiguous 0.26MB DMA, first on the ring) and tanh runs
on-device in small chunks so the first LDWEIGHTS issues as early as possible.

Schedule: m-half outer, then weight-stationary phases (blk, ncol), then four
1024-col PSUM tiles per phase-half (2 banks each, 4-tile rotation = all 8
banks). Each PSUM tile takes 4 matmuls (kc 0/1 accumulation x 2 chunks of
512) and is evacuated whole by DVE and ACT alternating per tile — the deep
rotation keeps PSUM recycling off the PE's critical path (a 2-deep rotation
measured ~0.8us PE stalls per store boundary). The first two x tiles are
split into 1024-col chunks so the first matmul isn't gated on a full 1MB
load. Loads own the Sync HWDGE ring; stores go on the ACT ring (the only
other HWDGE ring); the final store is split in two so the drain tail is
short.

Measured rel_l2 vs f32 reference: ~1.35e-2 (dominated by the e3m4 output
rounding; gate is 2e-2).
"""

import ml_dtypes
import numpy as np

import concourse.mybir as mybir
import concourse.tile as tile
from concourse import bacc
from concourse.bass_utils import run_bass_kernel_spmd

N_CORES = 8
N_ROWS = 8192            # rows of x / out
D = 4096                 # layer size
BLOCK = 256              # block size
BLOCKS_PER_CORE = 2      # 16 blocks / 8 cores
K_PER_CORE = BLOCKS_PER_CORE * BLOCK   # 512 k (and n) columns per core
MM_FREE = 512            # matmul moving free dim (one fp32 PSUM bank)
MSEG = 1024              # PSUM tile free dim (2 banks)
HALF = 4096              # store granularity / big x-tile free dim
OUT_SCALE = 8.0          # host folds x/8 in, decodes out*8

_nc_cache = None


def _build_nc():
    f32 = mybir.dt.float32
    bf16 = mybir.dt.bfloat16
    e3 = mybir.dt.float8e3

    # Bacc (not Bass): its compile() runs move_matmul_waits_to_ldweights and
    # generate_event_semaphores (splits multi-sem waits down to the 1
    # sync-wait-per-instruction the hardware supports).
    nc = bacc.Bacc("TRN2")
    xT = nc.dram_tensor("xT", [K_PER_CORE, N_ROWS], bf16, kind="ExternalInput")
    wp = nc.dram_tensor("wp", [128, 1024], bf16, kind="ExternalInput")
    outT = nc.dram_tensor("outT", [K_PER_CORE, N_ROWS], e3, kind="ExternalOutput")

    with tile.TileContext(nc) as tc:
        with (
            tc.tile_pool(name="bpool", bufs=1) as bpool,
            tc.tile_pool(name="xpool", bufs=14) as xpool,
            tc.tile_pool(name="opool", bufs=4) as opool,
            tc.tile_pool(name="pspool", bufs=4, space="PSUM") as pspool,
        ):
            # --- weights: single contiguous DMA of the host-packed lhsT
            # layout; col chunk (blk*2+kc)*256 + n holds
            # blocks[k0+blk*256+kc*128+p, k0+blk*256+n]
            w_raw = bpool.tile([128, 1024], bf16, name="w_raw")
            b_mm = bpool.tile([128, 1024], bf16, name="b_mm")
            nc.sync.dma_start(out=w_raw[:], in_=wp[:])

            # --- x loads on the Sync ring, in consumption order. The first
            # two tiles (q0/q1, first m-half) are split into 1024-col chunks
            # so the first matmuls aren't gated on a 1MB wire transfer.
            xts = {}

            def load(q, c0, c1, key):
                t = xpool.tile([128, c1 - c0], bf16, name=f"x{key}", tag="xt")
                nc.sync.dma_start(
                    out=t[:], in_=xT[q * 128:(q + 1) * 128, c0:c1]
                )
                xts[key] = t

            for c in range(4):
                load(0, c * MSEG, (c + 1) * MSEG, f"0s{c}")
                load(1, c * MSEG, (c + 1) * MSEG, f"1s{c}")
            load(2, 0, HALF, "2h0")
            load(3, 0, HALF, "3h0")
            for q in range(4):
                load(q, HALF, N_ROWS, f"{q}h1")

            def xslice(q, mlo):
                # [mlo, mlo+512) never straddles a tile boundary
                if q < 2 and mlo < HALF:
                    t = xts[f"{q}s{mlo // MSEG}"]
                    return t[:, mlo % MSEG:mlo % MSEG + MM_FREE]
                t = xts[f"{q}h{mlo // HALF}"]
                return t[:, mlo % HALF:mlo % HALF + MM_FREE]

            # tanh in dependency order: the (blk0, kc0/kc1, ncol0) lhsT
            # chunks first so LDWEIGHTS can start right after the table load
            for c0, c1 in [(0, 128), (256, 384), (128, 256), (384, 512),
                           (512, 1024)]:
                nc.scalar.activation(
                    b_mm[:, c0:c1], w_raw[:, c0:c1],
                    mybir.ActivationFunctionType.Tanh,
                )

            # --- matmul phases: m-half outer so the load stream keeps pace
            ms_idx = 0
            for h in range(2):
                for blk in range(BLOCKS_PER_CORE):
                    for ncol in range(2):  # n chunk of 128 within the block
                        r0 = blk * BLOCK + ncol * 128
                        osb = opool.tile([128, HALF], e3, name="osb",
                                         tag="osb")
                        for m4 in range(HALF // MSEG):
                            mlo0 = h * HALF + m4 * MSEG
                            ps = pspool.tile([128, MSEG], f32, name="ps")
                            for kc in range(2):
                                lcol = (blk * 2 + kc) * 256 + ncol * 128
                                lhsT = b_mm[:, lcol:lcol + 128]
                                for mi in range(MSEG // MM_FREE):
                                    nc.tensor.matmul(
                                        ps[:, mi * MM_FREE:(mi + 1) * MM_FREE],
                                        lhsT=lhsT,
                                        rhs=xslice(blk * 2 + kc,
                                                   mlo0 + mi * MM_FREE),
                                        start=(kc == 0),
                                        stop=(kc == 1),
                                    )
                            # whole-tile evac, DVE/ACT alternating: 4-deep
                            # PSUM rotation keeps recycling off the PE path
                            dst = osb[:, m4 * MSEG:(m4 + 1) * MSEG]
                            if ms_idx % 2 == 0:
                                nc.vector.tensor_copy(dst, ps[:])
                            else:
                                nc.scalar.copy(dst, ps[:])
                            ms_idx += 1
                        # stores on the ACT HWDGE ring (the only HWDGE ring
                        # besides Sync, which the loads own)
                        last = h == 1 and blk == 1 and ncol == 1
                        if not last:
                            nc.scalar.dma_start(
                                out=outT[r0:r0 + 128, h * HALF:(h + 1) * HALF],
                                in_=osb[:],
                            )
                        else:
                            # split the final store so the drain tail after
                            # the last evac is half as long
                            for s in range(2):
                                nc.scalar.dma_start(
                                    out=outT[r0:r0 + 128,
                                             h * HALF + s * 2048:
                                             h * HALF + (s + 1) * 2048],
                                    in_=osb[:, s * 2048:(s + 1) * 2048],
                                )
    nc.compile()
    return nc


def _get_nc():
    global _nc_cache
    if _nc_cache is None:
        _nc_cache = _build_nc()
    return _nc_cache


def _make_in_maps(x, blocks):
    bf = ml_dtypes.bfloat16
    # x/8 is exact in bf16 (exponent shift); the device computes out/8 so the
    # e3m4 output wire never saturates (|out/8| < 7.7 vs e3m4 max 15.5)
    xT = (x.T / OUT_SCALE).astype(bf)  # [4096, 8192]
    in_maps = []
    for c in range(N_CORES):
        k0 = c * K_PER_CORE
        cols = []
        for blk in range(BLOCKS_PER_CORE):
            for kc in range(2):
                rlo = k0 + blk * BLOCK + kc * 128
                cols.append(blocks[rlo:rlo + 128,
                                   k0 + blk * BLOCK:k0 + (blk + 1) * BLOCK])
        wpk = np.ascontiguousarray(np.concatenate(cols, axis=1)).astype(bf)
        in_maps.append({
            "xT": np.ascontiguousarray(xT[k0:k0 + K_PER_CORE]),
            "wp": wpk,
        })
    return in_maps


def _run(x, blocks, **spmd_kwargs):
    res = run_bass_kernel_spmd(
        _get_nc(), _make_in_maps(x, blocks), core_ids=list(range(N_CORES)),
        **spmd_kwargs,
    )
    out = np.empty((N_ROWS, D), np.float32)
    for c in range(N_CORES):
        shard = res.results[c]["outT"].astype(np.float32) * OUT_SCALE
        out[:, c * K_PER_CORE:(c + 1) * K_PER_CORE] = shard.T
    return out, res


def kernel(x, blocks, mask=None):
    out, _ = _run(np.asarray(x), np.asarray(blocks))
    return out
